# revision 20
# baseline (speedup 1.0000x reference)
"""Trainium2 Bass kernel for DeepgazeSpadeV2 segment_reduce (plan A polish).

bf16 paint matmuls (1 cyc/col), host-built bf16 one-hot DMA'd in (frees
DVE/Pool of all one-hot work), PSUM->SBUF bf16 copies split ACT/DVE,
output DMA as bf16 on TWO HWDGE queues, host upcasts to fp32.
"""

import sys

if "/opt/trn_rl_repo" not in sys.path:
    sys.path.insert(0, "/opt/trn_rl_repo")

import numpy as np
import ml_dtypes

B, C, HP, WP = 2, 768, 18, 18
HI, WI = 256, 256
S = 256
NP_PATCH = HP * WP
P_PAD = 384
N_CORES = 8
SLICES_PER_BATCH = N_CORES // B
ROWS_PER_SLICE = HI // SLICES_PER_BATCH   # 64
NPIX = ROWS_PER_SLICE * WI                # 16384
GPIX = 2048                               # pixels per psum group (4 banks)
NGRP = NPIX // GPIX                       # 8
SUB = GPIX // 512                         # 4
CT = C // 128                             # 6

_CACHE = {}


def _build():
    import concourse.bacc as bacc
    import concourse.mybir as mybir
    from concourse.tile import TileContext

    f32 = mybir.dt.float32
    bf16 = mybir.dt.bfloat16
    EQ = mybir.AluOpType.is_equal
    MULT = mybir.AluOpType.mult

    nc = bacc.Bacc("TRN2", target_bir_lowering=False, debug=False)
    featsT = nc.dram_tensor("featsT", [P_PAD, C], bf16, kind="ExternalInput")
    segp = nc.dram_tensor("segp", [P_PAD], f32, kind="ExternalInput")
    ohbig = nc.dram_tensor("ohbig", [128, 2 * NPIX], bf16, kind="ExternalInput")
    out = nc.dram_tensor("out", [C, NPIX], bf16, kind="ExternalOutput")

    with TileContext(nc) as tc:
        with (
            tc.tile_pool(name="const", bufs=1) as cp,
            tc.tile_pool(name="work", bufs=3) as wp,
            tc.tile_pool(name="stage", bufs=2) as sp,
        ):
            # host-built one-hot [p, st, pix], loaded as one tile PER GROUP so
            # the first paint matmuls only wait on their own 1MB chunk (a
            # single 8MB load stalls the whole paint ~24us on its semaphore)
            ohsrc = ohbig.ap().rearrange("p (st gg n) -> p st gg n", st=2, gg=NGRP)
            ohb_g = []
            for g in range(NGRP):
                t = cp.tile([128, 2, GPIX], bf16, tag=f"ohb{g}")
                nc.sync.dma_start(out=t[:, :, :], in_=ohsrc[:, :, g, :])
                ohb_g.append(t)

            # PE warmup: trip the HAM clock gate
            psA_cm0 = tc.tile_pool(name="psW", bufs=1, space="PSUM")
            psW = psA_cm0.__enter__()
            warm_w = cp.tile([128, 64], bf16, tag="warm_w")
            nc.any.memset(warm_w[:, :], 1.0)
            warm_x = cp.tile([128, 64], bf16, tag="warm_x")
            nc.any.memset(warm_x[:, :], 1.0)
            ps_warm = psW.tile([64, 64], f32, tag="warm")
            NWARM = 60
            for i in range(NWARM):
                nc.tensor.matmul(
                    ps_warm[:, :], warm_w[:, :], warm_x[:, :],
                    start=(i == 0), stop=(i == NWARM - 1),
                )
            psA_cm0.__exit__(None, None, None)

            ft = cp.tile([128, 3, C], bf16, tag="ft")
            ftr = featsT.ap().rearrange("(k p) c -> p k c", p=128)
            for k in range(3):
                nc.sync.dma_start(out=ft[:, k, :], in_=ftr[:, k, :])
            sp_f = cp.tile([128, 3], f32, tag="sp_f")
            nc.sync.dma_start(out=sp_f[:, :], in_=segp.ap().rearrange("(k p) -> p k", p=128))

            io_f = cp.tile([128, S], f32, tag="io_f")
            nc.gpsimd.iota(io_f[:, :], pattern=[[1, S]], base=0, channel_multiplier=0,
                           allow_small_or_imprecise_dtypes=True)

            ones_col = cp.tile([128, 1], bf16, tag="ones_col")
            nc.any.memset(ones_col[:, :], 1.0)

            ohp = cp.tile([128, 3, S], bf16, tag="ohp")
            for k in range(3):
                nc.vector.tensor_scalar(ohp[:, k, :], io_f[:, :], sp_f[:, k : k + 1], None, EQ)

            psA_cm = tc.tile_pool(name="psA", bufs=2, space="PSUM")
            psA = psA_cm.__enter__()
            recip = cp.tile([128, 2], f32, tag="recip")
            for st in range(2):
                ps_cnt = psA.tile([128, 1], f32, tag="cnt")
                for k in range(3):
                    nc.tensor.matmul(
                        ps_cnt[:, :],
                        ohp[:, k, st * 128 : (st + 1) * 128],
                        ones_col[:, :],
                        start=(k == 0),
                        stop=(k == 2),
                    )
                cnt_cl = wp.tile([128, 1], f32, tag="cnt_cl")
                nc.vector.tensor_scalar_max(cnt_cl[:, :], ps_cnt[:, :], 1.0)
                nc.vector.reciprocal(recip[:, st : st + 1], cnt_cl[:, :])

            tab = cp.tile([128, 2, C], bf16, tag="tab")
            for st in range(2):
                for cc in range(2):
                    ps_sum = psA.tile([128, 384], f32, tag="sums")
                    for k in range(3):
                        nc.tensor.matmul(
                            ps_sum[:, :],
                            ohp[:, k, st * 128 : (st + 1) * 128],
                            ft[:, k, cc * 384 : (cc + 1) * 384],
                            start=(k == 0),
                            stop=(k == 2),
                        )
                    nc.vector.tensor_scalar(
                        tab[:, st, cc * 384 : (cc + 1) * 384],
                        ps_sum[:, :],
                        recip[:, st : st + 1],
                        None,
                        MULT,
                    )
            psA_cm.__exit__(None, None, None)

            # ---- paint ----
            copy_flip = [0]
            with tc.tile_pool(name="psB", bufs=2, space="PSUM") as psB:
                for g in range(NGRP):
                    stages = [
                        sp.tile([128, GPIX], bf16, tag=f"stg{ct}", name=f"stg{ct}")
                        for ct in range(CT)
                    ]
                    for ct in range(CT):
                        ps_o = psB.tile([128, GPIX], f32, tag="out")
                        for k in range(2):
                            for jj in range(SUB):
                                nc.tensor.matmul(
                                    ps_o[:, jj * 512 : (jj + 1) * 512],
                                    tab[:, k, ct * 128 : (ct + 1) * 128],
                                    ohb_g[g][:, k, jj * 512 : (jj + 1) * 512],
                                    start=(k == 0),
                                    stop=(k == 1),
                                    skip_group_check=True,
                                )
                        if copy_flip[0] % 2 == 0:
                            nc.scalar.copy(out=stages[ct][:, :], in_=ps_o[:, :])
                        else:
                            nc.vector.tensor_copy(stages[ct][:, :], ps_o[:, :])
                        copy_flip[0] += 1
                        dma_eng = nc.sync if ct % 2 == 0 else nc.scalar
                        dma_eng.dma_start(
                            out=out.ap()[
                                ct * 128 : (ct + 1) * 128,
                                g * GPIX : (g + 1) * GPIX,
                            ],
                            in_=stages[ct][:, :],
                        )
    nc.compile()
    return nc


def _get_nc():
    if "nc" not in _CACHE:
        _CACHE["nc"] = _build()
    return _CACHE["nc"]


def _make_in_maps(feats, segmap):
    idx_h = (np.arange(HP) * HI) // HP
    idx_w = (np.arange(WP) * WI) // WP
    grid = np.arange(128)[:, None, None] + 128 * np.arange(2)[None, :, None]
    in_maps = []
    for core in range(N_CORES):
        b = core // SLICES_PER_BATCH
        q = core % SLICES_PER_BATCH
        ftp = np.zeros((P_PAD, C), dtype=ml_dtypes.bfloat16)
        ftp[:NP_PATCH] = feats[b].reshape(C, NP_PATCH).T.astype(ml_dtypes.bfloat16)
        spp = np.full((P_PAD,), S, dtype=np.float32)
        seg_b = np.clip(segmap[b], 0, S - 1)
        spp[:NP_PATCH] = seg_b[idx_h[:, None], idx_w[None, :]].reshape(-1).astype(np.float32)
        pix = seg_b[q * ROWS_PER_SLICE : (q + 1) * ROWS_PER_SLICE, :].reshape(-1)
        # one-hot [p, st, n] = (pix[n] == 128*st + p), bf16
        ohb = (pix[None, None, :] == grid).astype(ml_dtypes.bfloat16)
        in_maps.append(
            {
                "featsT": ftp,
                "segp": spp,
                "ohbig": ohb.reshape(128, 2 * NPIX),
            }
        )
    return in_maps


def _run(in_maps, **kwargs):
    from concourse.bass_utils import run_bass_kernel_spmd

    nc = _get_nc()
    return run_bass_kernel_spmd(nc, in_maps, core_ids=list(range(N_CORES)), **kwargs)


def kernel(feats, segmap, num_total_segments):
    feats = np.asarray(feats, dtype=np.float32)
    segmap = np.asarray(segmap, dtype=np.int32)
    assert int(num_total_segments) == S
    assert feats.shape == (B, C, HP, WP) and segmap.shape == (B, HI, WI)

    res = _run(_make_in_maps(feats, segmap))
    out = np.empty((B, C, HI, WI), dtype=np.float32)
    for core in range(N_CORES):
        b = core // SLICES_PER_BATCH
        q = core % SLICES_PER_BATCH
        out[b, :, q * ROWS_PER_SLICE : (q + 1) * ROWS_PER_SLICE, :] = (
            res.results[core]["out"].astype(np.float32).reshape(C, ROWS_PER_SLICE, WI)
        )
    return out


# revision 22
# speedup vs baseline: 1.1681x; 1.1681x over previous
"""Trainium2 Bass kernel for DeepgazeSpadeV2 segment_reduce (plan A polish).

bf16 paint matmuls (1 cyc/col), host-built bf16 one-hot DMA'd in (frees
DVE/Pool of all one-hot work), PSUM->SBUF bf16 copies split ACT/DVE,
output DMA as bf16 on TWO HWDGE queues, host upcasts to fp32.
"""

import sys

if "/opt/trn_rl_repo" not in sys.path:
    sys.path.insert(0, "/opt/trn_rl_repo")

import numpy as np
import ml_dtypes

B, C, HP, WP = 2, 768, 18, 18
HI, WI = 256, 256
S = 256
NP_PATCH = HP * WP
P_PAD = 384
N_CORES = 8
SLICES_PER_BATCH = N_CORES // B
ROWS_PER_SLICE = HI // SLICES_PER_BATCH   # 64
NPIX = ROWS_PER_SLICE * WI                # 16384
GPIX = 2048                               # pixels per psum group (4 banks)
NGRP = NPIX // GPIX                       # 8
SUB = GPIX // 512                         # 4
CT = C // 128                             # 6

_CACHE = {}


def _build():
    import concourse.bacc as bacc
    import concourse.mybir as mybir
    from concourse.tile import TileContext

    f32 = mybir.dt.float32
    bf16 = mybir.dt.bfloat16
    EQ = mybir.AluOpType.is_equal
    MULT = mybir.AluOpType.mult

    nc = bacc.Bacc("TRN2", target_bir_lowering=False, debug=False)
    featsT = nc.dram_tensor("featsT", [P_PAD, C], bf16, kind="ExternalInput")
    segp = nc.dram_tensor("segp", [P_PAD], f32, kind="ExternalInput")
    ohbig = nc.dram_tensor("ohbig", [128, 2 * NPIX], bf16, kind="ExternalInput")
    out = nc.dram_tensor("out", [C, NPIX], bf16, kind="ExternalOutput")

    with TileContext(nc) as tc:
        with (
            tc.tile_pool(name="const", bufs=1) as cp,
            tc.tile_pool(name="work", bufs=3) as wp,
            tc.tile_pool(name="stage", bufs=2) as sp,
        ):
            # PE warmup: trip the HAM clock gate
            psA_cm0 = tc.tile_pool(name="psW", bufs=1, space="PSUM")
            psW = psA_cm0.__enter__()
            warm_w = cp.tile([128, 64], bf16, tag="warm_w")
            nc.any.memset(warm_w[:, :], 1.0)
            warm_x = cp.tile([128, 64], bf16, tag="warm_x")
            nc.any.memset(warm_x[:, :], 1.0)
            ps_warm = psW.tile([64, 64], f32, tag="warm")
            NWARM = 60
            for i in range(NWARM):
                nc.tensor.matmul(
                    ps_warm[:, :], warm_w[:, :], warm_x[:, :],
                    start=(i == 0), stop=(i == NWARM - 1),
                )
            psA_cm0.__exit__(None, None, None)

            ft = cp.tile([128, 3, C], bf16, tag="ft")
            ftr = featsT.ap().rearrange("(k p) c -> p k c", p=128)
            for k in range(3):
                nc.sync.dma_start(out=ft[:, k, :], in_=ftr[:, k, :])
            sp_f = cp.tile([128, 3], f32, tag="sp_f")
            nc.sync.dma_start(out=sp_f[:, :], in_=segp.ap().rearrange("(k p) -> p k", p=128))

            # host-built one-hot [p, st, pix], loaded as one tile PER GROUP
            # (per-tile semaphores: paint group g waits only on its own 1MB
            # chunk). Emitted AFTER ft/sp_f so phase A's inputs aren't queued
            # behind 8MB of one-hot on the sync DGE.
            ohsrc = ohbig.ap().rearrange("p (st gg n) -> p st gg n", st=2, gg=NGRP)
            ohb_g = []
            for g in range(NGRP):
                t = cp.tile([128, 2, GPIX], bf16, tag=f"ohb{g}")
                nc.sync.dma_start(out=t[:, :, :], in_=ohsrc[:, :, g, :])
                ohb_g.append(t)

            io_f = cp.tile([128, S], f32, tag="io_f")
            nc.gpsimd.iota(io_f[:, :], pattern=[[1, S]], base=0, channel_multiplier=0,
                           allow_small_or_imprecise_dtypes=True)

            ones_col = cp.tile([128, 1], bf16, tag="ones_col")
            nc.any.memset(ones_col[:, :], 1.0)

            ohp = cp.tile([128, 3, S], bf16, tag="ohp")
            for k in range(3):
                nc.vector.tensor_scalar(ohp[:, k, :], io_f[:, :], sp_f[:, k : k + 1], None, EQ)

            psA_cm = tc.tile_pool(name="psA", bufs=2, space="PSUM")
            psA = psA_cm.__enter__()
            recip = cp.tile([128, 2], f32, tag="recip")
            for st in range(2):
                ps_cnt = psA.tile([128, 1], f32, tag="cnt")
                for k in range(3):
                    nc.tensor.matmul(
                        ps_cnt[:, :],
                        ohp[:, k, st * 128 : (st + 1) * 128],
                        ones_col[:, :],
                        start=(k == 0),
                        stop=(k == 2),
                    )
                cnt_cl = wp.tile([128, 1], f32, tag="cnt_cl")
                nc.vector.tensor_scalar_max(cnt_cl[:, :], ps_cnt[:, :], 1.0)
                nc.vector.reciprocal(recip[:, st : st + 1], cnt_cl[:, :])

            tab = cp.tile([128, 2, C], bf16, tag="tab")
            for st in range(2):
                for cc in range(2):
                    ps_sum = psA.tile([128, 384], f32, tag="sums")
                    for k in range(3):
                        nc.tensor.matmul(
                            ps_sum[:, :],
                            ohp[:, k, st * 128 : (st + 1) * 128],
                            ft[:, k, cc * 384 : (cc + 1) * 384],
                            start=(k == 0),
                            stop=(k == 2),
                        )
                    nc.vector.tensor_scalar(
                        tab[:, st, cc * 384 : (cc + 1) * 384],
                        ps_sum[:, :],
                        recip[:, st : st + 1],
                        None,
                        MULT,
                    )
            psA_cm.__exit__(None, None, None)

            # ---- paint ----
            copy_flip = [0]
            with tc.tile_pool(name="psB", bufs=2, space="PSUM") as psB:
                for g in range(NGRP):
                    stages = [
                        sp.tile([128, GPIX], bf16, tag=f"stg{ct}", name=f"stg{ct}")
                        for ct in range(CT)
                    ]
                    for ct in range(CT):
                        ps_o = psB.tile([128, GPIX], f32, tag="out")
                        for k in range(2):
                            for jj in range(SUB):
                                nc.tensor.matmul(
                                    ps_o[:, jj * 512 : (jj + 1) * 512],
                                    tab[:, k, ct * 128 : (ct + 1) * 128],
                                    ohb_g[g][:, k, jj * 512 : (jj + 1) * 512],
                                    start=(k == 0),
                                    stop=(k == 1),
                                    skip_group_check=True,
                                )
                        if copy_flip[0] % 2 == 0:
                            nc.scalar.copy(out=stages[ct][:, :], in_=ps_o[:, :])
                        else:
                            nc.vector.tensor_copy(stages[ct][:, :], ps_o[:, :])
                        copy_flip[0] += 1
                        dma_eng = nc.sync if ct % 2 == 0 else nc.scalar
                        dma_eng.dma_start(
                            out=out.ap()[
                                ct * 128 : (ct + 1) * 128,
                                g * GPIX : (g + 1) * GPIX,
                            ],
                            in_=stages[ct][:, :],
                        )
    nc.compile()
    return nc


def _get_nc():
    if "nc" not in _CACHE:
        _CACHE["nc"] = _build()
    return _CACHE["nc"]


def _make_in_maps(feats, segmap):
    idx_h = (np.arange(HP) * HI) // HP
    idx_w = (np.arange(WP) * WI) // WP
    grid = np.arange(128)[:, None, None] + 128 * np.arange(2)[None, :, None]
    in_maps = []
    for core in range(N_CORES):
        b = core // SLICES_PER_BATCH
        q = core % SLICES_PER_BATCH
        ftp = np.zeros((P_PAD, C), dtype=ml_dtypes.bfloat16)
        ftp[:NP_PATCH] = feats[b].reshape(C, NP_PATCH).T.astype(ml_dtypes.bfloat16)
        spp = np.full((P_PAD,), S, dtype=np.float32)
        seg_b = np.clip(segmap[b], 0, S - 1)
        spp[:NP_PATCH] = seg_b[idx_h[:, None], idx_w[None, :]].reshape(-1).astype(np.float32)
        pix = seg_b[q * ROWS_PER_SLICE : (q + 1) * ROWS_PER_SLICE, :].reshape(-1)
        # one-hot [p, st, n] = (pix[n] == 128*st + p), bf16
        ohb = (pix[None, None, :] == grid).astype(ml_dtypes.bfloat16)
        in_maps.append(
            {
                "featsT": ftp,
                "segp": spp,
                "ohbig": ohb.reshape(128, 2 * NPIX),
            }
        )
    return in_maps


def _run(in_maps, **kwargs):
    from concourse.bass_utils import run_bass_kernel_spmd

    nc = _get_nc()
    return run_bass_kernel_spmd(nc, in_maps, core_ids=list(range(N_CORES)), **kwargs)


def kernel(feats, segmap, num_total_segments):
    feats = np.asarray(feats, dtype=np.float32)
    segmap = np.asarray(segmap, dtype=np.int32)
    assert int(num_total_segments) == S
    assert feats.shape == (B, C, HP, WP) and segmap.shape == (B, HI, WI)

    res = _run(_make_in_maps(feats, segmap))
    out = np.empty((B, C, HI, WI), dtype=np.float32)
    for core in range(N_CORES):
        b = core // SLICES_PER_BATCH
        q = core % SLICES_PER_BATCH
        out[b, :, q * ROWS_PER_SLICE : (q + 1) * ROWS_PER_SLICE, :] = (
            res.results[core]["out"].astype(np.float32).reshape(C, ROWS_PER_SLICE, WI)
        )
    return out
